# revision 64
# baseline (speedup 1.0000x reference)
"""GPT forward (8 layers, C=1024, T=1024, B=2, H=16, V=32000) on 8 trn2 cores.

Sharding: TP4 x DP2. Cores 0-3 handle batch 0, cores 4-7 batch 1.
Within a quad, core j owns heads 4j..4j+3, MLP hidden slice j*1024..,
and vocab slice j*8000.. of the LM head.

v4 design:
- LN gamma/beta folded into weights/biases on HOST (exact linear algebra):
  W' = diag(gamma) @ W, b' = b + W^T beta. Device LN work is only stats
  (mean / mean-square, bf16 matmuls with a ones/C stationary vector) plus
  rstd scaling fused into each matmul epilogue.
- Mean & bias handled by a 2-row augmented matmul appended to each
  accumulation group: psum += [-colsum(W'); b']^T @ [mu; std], so
  out = psum * rstd equals W'(x-mu)rstd + b' exactly.
- Software-pipelined LN: the bf16 copy of the residual is produced BY the
  residual-update op itself (second DVE op reading the same inputs), and
  LN stats run at the tail of the producing phase.
- Token-phase split (two halves of T); chain consumers (residual adds) are
  issued AFTER the other phase's attention so the per-engine FIFOs never
  head-of-line block independent work behind a DMA wait.
- Softmax denominator broadcast runs on the PE (ones-column matmul into
  the same PSUM tile as att@V) - Pool stays off the attention path.
- Fine-grained causal skipping in scores AND in att@V (ragged psum).
- Weight loads stream on the Act queue, prefetched as soon as the
  previous layer's readers release the buffers.
- All collective traffic bf16; merged big-DMA layouts for the MLP-reduce
  readback, LM head weights and logits output (host-side relayout).
"""

import numpy as np
import ml_dtypes

import concourse.bacc as bacc
import concourse.bass as bass
import concourse.tile as tile
import concourse.mybir as mybir
from concourse import bass_utils

f32 = mybir.dt.float32
bf16 = mybir.dt.bfloat16
AF = mybir.ActivationFunctionType
OP = mybir.AluOpType

B, T, C, L, H, F, V = 2, 1024, 1024, 8, 16, 4096, 32000
HD = C // H            # 64
TP = 4                 # tensor-parallel within a quad
HL = H // TP           # 4 local heads
QO = C // TP           # 256 local q/k/v width
FL = F // TP           # 1024 local mlp hidden
VL = V // TP           # 8000 local vocab
VLP = 8192             # padded local vocab (16 x 512)
NVB = VLP // 512       # 16 vocab blocks
NCH = C // 128         # 8 channel chunks
NTC = T // 128         # 8 token chunks
TH = 512               # token phase size
GROUPS = [[0, 1, 2, 3], [4, 5, 6, 7]]
LN_EPS = 1e-5
SCALE = 1.0 / np.sqrt(HD)

_STATE = {}


def _build(collectives=True, zero_bias=False):
    nc = bacc.Bacc("TRN2", target_bir_lowering=False, debug=False,
                   enable_asserts=False, num_devices=8)

    x0T_d = nc.dram_tensor("x0t", [C, T], f32, kind="ExternalInput").ap()
    x0b_d = nc.dram_tensor("x0b", [C, T], bf16, kind="ExternalInput").ap()
    wqkv_d = nc.dram_tensor("wqkv", [L, C, 3 * QO], bf16, kind="ExternalInput").ap()
    w1_d = nc.dram_tensor("w1", [L, C, FL], bf16, kind="ExternalInput").ap()
    w2_d = nc.dram_tensor("w2", [L, FL, C], bf16, kind="ExternalInput").ap()
    augw_d = nc.dram_tensor("augw", [L, 2, 3 * QO + FL], bf16,
                            kind="ExternalInput").ap()
    b2_d = nc.dram_tensor("b2c", [128, L * 8], f32, kind="ExternalInput").ap()
    # head weights, host-relaid: hw[p, vb, cc, q] = hw_g[cc*128+p, vb*512+q]
    hw_d = nc.dram_tensor("hw", [128, NVB, NCH, 512], bf16,
                          kind="ExternalInput").ap()
    mask_d = nc.dram_tensor("mask", [128, 128], bf16, kind="ExternalInput").ap()
    # logits, host-relaid: out[p, tcc, vb, q] = logit[tcc*128+p, vb*512+q]
    out_d = nc.dram_tensor("out", [128, NTC, NVB, 512], bf16,
                           kind="ExternalOutput").ap()

    with tile.TileContext(nc) as tc:
        _prog(nc, tc, x0T_d, x0b_d, wqkv_d, w1_d, w2_d, augw_d, b2_d, hw_d,
              mask_d, out_d, collectives, zero_bias)
    nc.compile()
    return nc


def _prog(nc, tc, x0T_d, x0b_d, wqkv_d, w1_d, w2_d, augw_d, b2_d, hw_d,
          mask_d, out_d, collectives=True, zero_bias=False):
    import contextlib
    ctx = contextlib.ExitStack()
    with ctx:
        const = ctx.enter_context(tc.tile_pool(name="const", bufs=1))
        xp = ctx.enter_context(tc.tile_pool(name="xres", bufs=16))
        xbp = ctx.enter_context(tc.tile_pool(name="xb", bufs=16))
        hbp = ctx.enter_context(tc.tile_pool(name="hb", bufs=9))
        sqp = ctx.enter_context(tc.tile_pool(name="sq", bufs=7))
        qkp = ctx.enter_context(tc.tile_pool(name="qk", bufs=8))
        vp = ctx.enter_context(tc.tile_pool(name="vsb", bufs=8))
        attp = ctx.enter_context(tc.tile_pool(name="att", bufs=7))
        yp = ctx.enter_context(tc.tile_pool(name="ysb", bufs=4))
        rbp = ctx.enter_context(tc.tile_pool(name="rbk", bufs=8))
        gap = ctx.enter_context(tc.tile_pool(name="ga", bufs=8))
        rsp = ctx.enter_context(tc.tile_pool(name="rs", bufs=2))
        wqp = ctx.enter_context(tc.tile_pool(name="wq", bufs=8))
        w1p = ctx.enter_context(tc.tile_pool(name="w1", bufs=8))
        w2p = ctx.enter_context(tc.tile_pool(name="w2", bufs=8))
        awp = ctx.enter_context(tc.tile_pool(name="aw", bufs=1))
        a2p = ctx.enter_context(tc.tile_pool(name="a2", bufs=2))
        bcp = ctx.enter_context(tc.tile_pool(name="bc", bufs=4))
        smp = ctx.enter_context(tc.tile_pool(name="sm", bufs=3))
        dbp = ctx.enter_context(tc.tile_pool(name="db", bufs=2))
        rcp = ctx.enter_context(tc.tile_pool(name="rc", bufs=3))
        hwp = ctx.enter_context(tc.tile_pool(name="hww", bufs=3))
        sop = ctx.enter_context(tc.tile_pool(name="so", bufs=2))
        psb = ctx.enter_context(tc.tile_pool(name="psb", bufs=4, space="PSUM"))
        pst = ctx.enter_context(tc.tile_pool(name="pst", bufs=1, space="PSUM"))
        pav = ctx.enter_context(tc.tile_pool(name="pav", bufs=2, space="PSUM"))
        dr = ctx.enter_context(tc.tile_pool(name="dram", bufs=4, space="DRAM"))

        onesb = const.tile([128, 1], bf16)
        nc.vector.memset(onesb[:], 1.0 / C)
        eps_t = const.tile([1, 1], f32, tag="eps")
        nc.vector.memset(eps_t[:], LN_EPS)
        mask = const.tile([128, 128], bf16, tag="mask")
        nc.sync.dma_start(mask[:], mask_d[:])
        b2c = const.tile([128, L * 8], f32, tag="b2c")
        nc.sync.dma_start(b2c[:], b2_d[:])

        # residual stream: 16 persistent fp32 tiles [128 ch, 512 tok],
        # upcast from the host-cast bf16 x0 (loaded below)
        xt = [[None] * 2 for _ in range(NCH)]
        for cc in range(NCH):
            for th in range(2):
                t = xp.tile([128, TH], f32)
                xt[cc][th] = t

        # persistent v tiles [128 tok, 4*(HD+1)]: ones cols set once
        v_t = []
        for tcc in range(NTC):
            vt = vp.tile([128, 4 * (HD + 1)], bf16)
            for hh in range(HL):
                nc.vector.memset(vt[:, hh * 65 + 64:hh * 65 + 65], 1.0)
            v_t.append(vt)

        def ln_stats(src_tiles, want_col):
            """src: 8 bf16 [128,512] tiles. Returns dict with aug2 [2,512]
            bf16, rstd_b [128,512] f32 bcast, optional rstd col [128,4]."""
            mu_ps = pst.tile([1, TH], f32, tag="mu")
            msq_ps = pst.tile([1, TH], f32, tag="msq")
            sqs = []
            for cc in range(NCH):
                sq = sqp.tile([128, TH], bf16, tag="sq", name="sq")
                eng = nc.vector if cc % 2 else nc.gpsimd
                eng.tensor_mul(sq[:], src_tiles[cc][:], src_tiles[cc][:])
                sqs.append(sq)
            for cc in range(NCH):
                nc.tensor.matmul(mu_ps[:], onesb[:], src_tiles[cc][:],
                                 start=(cc == 0), stop=(cc == NCH - 1))
            for cc in range(NCH):
                nc.tensor.matmul(msq_ps[:], onesb[:], sqs[cc][:],
                                 start=(cc == 0), stop=(cc == NCH - 1))
            mu_bf = a2p.tile([1, TH], bf16, tag="mu_bf")
            nc.scalar.activation(mu_bf[:], mu_ps[:], AF.Copy)
            mu2 = smp.tile([1, TH], f32, tag="sm")
            nc.vector.tensor_mul(mu2[:], mu_bf[:], mu_bf[:])
            var = smp.tile([1, TH], f32, tag="sm")
            nc.vector.scalar_tensor_tensor(var[:], msq_ps[:], 1.0, mu2[:],
                                           op0=OP.mult, op1=OP.subtract)
            std = smp.tile([1, TH], f32, tag="sm")
            nc.scalar.activation(std[:], var[:], AF.Sqrt, bias=eps_t[:])
            rstd = smp.tile([1, TH], f32, tag="sm")
            nc.vector.reciprocal(rstd[:], std[:])
            std_bf = None
            if not zero_bias:
                std_bf = a2p.tile([1, TH], bf16, tag="std_bf")
                nc.scalar.activation(std_bf[:], std[:], AF.Copy)
            rstd_b = bcp.tile([128, TH], f32, tag="bc128")
            nc.gpsimd.partition_broadcast(rstd_b[:], rstd[:])
            info = {"mu": mu_bf, "std": std_bf, "rstd_b": rstd_b}
            if want_col:
                rd = dr.tile([1, TH], f32, tag="rd")
                nc.sync.dma_start(rd[:], rstd[:])
                col = smp.tile([128, 4], f32, tag="col")
                nc.sync.dma_start(
                    col[:], rd[0:1, :].rearrange("a (j p) -> p (a j)", p=128))
                info["col"] = col
            return info

        def load_wq(l):
            wq_t = []
            for cc in range(NCH):
                t = wqp.tile([128, 3 * QO], bf16, tag="wq", name=f"wq{l}_{cc}")
                nc.sync.dma_start(t[:], wqkv_d[l, cc * 128:(cc + 1) * 128, :])
                wq_t.append(t)
            return wq_t

        def load_augw(l):
            augw = awp.tile([1, 2 * (3 * QO + FL)], bf16, tag="augw",
                            name=f"aug{l}")
            nc.sync.dma_start(augw[:], augw_d[l, :, :])
            # [0, 0:1792] = -colsum(W'); [0, 1792:3584] = b'
            return augw

        def load_w1(l):
            w1_t = []
            for cc in range(NCH):
                t = w1p.tile([128, FL], bf16, tag="w1", name=f"w1{l}_{cc}")
                nc.sync.dma_start(t[:], w1_d[l, cc * 128:(cc + 1) * 128, :])
                w1_t.append(t)
            return w1_t

        def load_w2(l):
            w2_t = []
            for cc in range(NCH):
                t = w2p.tile([128, C], bf16, tag="w2", name=f"w2{l}_{cc}")
                nc.sync.dma_start(t[:], w2_d[l, cc * 128:(cc + 1) * 128, :])
                w2_t.append(t)
            return w2_t

        AW = 3 * QO + FL   # 1792: offset of b' row inside augw

        # prologue: host-cast bf16 x0; f32 residual upcast on idle engines
        ln1 = [None, None]
        xb = [[None] * 2 for _ in range(NCH)]
        for th in range(2):
            for cc in range(NCH):
                t = xbp.tile([128, TH], bf16, tag="xb", name=f"xb0_{th}_{cc}")
                nc.sync.dma_start(t[:], x0b_d[cc * 128:(cc + 1) * 128,
                                              th * TH:(th + 1) * TH])
                xb[cc][th] = t
                if cc % 2:
                    nc.scalar.activation(xt[cc][th][:], t[:], AF.Copy)
                else:
                    nc.gpsimd.tensor_copy(xt[cc][th][:], t[:])
            ln1[th] = ln_stats([xb[cc][th] for cc in range(NCH)], want_col=True)

        wq_t = load_wq(0)
        augw = load_augw(0)
        w1_t = load_w1(0)
        w2_t = load_w2(0)
        for l in range(L):
            # ---- QKV projections (transposed layout [qo, 512] per phase)
            qk_t = [[None] * 2 for _ in range(4)]
            for th in range(2):
                for oc in range(4):
                    p = psb.tile([128, TH], f32, tag="psb", name="p")
                    for cc in range(NCH):
                        nc.tensor.matmul(p[:], wq_t[cc][:, oc * 128:(oc + 1) * 128],
                                         xb[cc][th][:],
                                         start=(cc == 0), stop=False)
                    nc.tensor.matmul(p[:], augw[0:1, oc * 128:(oc + 1) * 128],
                                     ln1[th]["mu"][:], start=False,
                                     stop=zero_bias)
                    if not zero_bias:
                        nc.tensor.matmul(
                            p[:], augw[0:1, AW + oc * 128:AW + (oc + 1) * 128],
                            ln1[th]["std"][:], start=False, stop=True)
                    dst = qkp.tile([128, TH], bf16, tag="qk", name="dst")
                    nc.vector.tensor_mul(dst[:], p[:], ln1[th]["rstd_b"][:])
                    qk_t[oc][th] = dst

            # ---- V (normal layout [tok, vo]), 4 heads packed + ones cols
            def v_phase(tccs):
              for tcc in tccs:
                th = tcc // 4
                pv = psb.tile([128, QO], f32, tag="psb", name="pv")
                for cc in range(NCH):
                    nc.tensor.matmul(
                        pv[:], xb[cc][th][:, (tcc % 4) * 128:(tcc % 4 + 1) * 128],
                        wq_t[cc][:, 2 * QO:3 * QO],
                        start=(cc == 0), stop=False)
                nc.tensor.matmul(
                    pv[:], ln1[th]["mu"][0:1, (tcc % 4) * 128:(tcc % 4 + 1) * 128],
                    augw[0:1, 2 * QO:3 * QO], start=False, stop=zero_bias)
                if not zero_bias:
                    nc.tensor.matmul(
                        pv[:],
                        ln1[th]["std"][0:1, (tcc % 4) * 128:(tcc % 4 + 1) * 128],
                        augw[0:1, AW + 2 * QO:AW + 3 * QO],
                        start=False, stop=True)
                for hh in range(HL):
                    nc.vector.tensor_scalar_mul(
                        v_t[tcc][:, hh * 65:hh * 65 + 64],
                        pv[:, hh * HD:(hh + 1) * HD],
                        ln1[th]["col"][:, tcc % 4:tcc % 4 + 1])

            v_phase(range(4))

            # ---- attention; chains issued immediately after each phase,
            # chain CONSUMERS (residual adds) issued after the other phase
            y_sb = [[yp.tile([128, TH], bf16, tag="y", name=f"ysb{l}_{i}_{th}")
                     for th in range(2)]
                    for i in range(2)]
            h2 = [[None] * 2 for _ in range(NCH)]
            yts = [[None] * NCH, [None] * NCH]

            def heads_phase(th, inject=None):
                q_lo = th * TH
                n_si = 4 if th == 0 else 8
                for hh in range(HL):
                    if inject and hh in inject:
                        inject[hh]()
                    qi, ro = hh // 2, (hh % 2) * 64
                    att = []
                    for si in range(n_si):
                        sc = si * 128
                        k_th, k_off = si // 4, (si % 4) * 128
                        lo = max(sc, q_lo) - q_lo   # local col offset
                        pa = psb.tile([128, TH], f32, tag="psb", name="pa")
                        nc.tensor.matmul(
                            pa[:, lo:TH],
                            qk_t[2 + qi][k_th][ro:ro + 64, k_off:k_off + 128],
                            qk_t[qi][th][ro:ro + 64, lo:TH],
                            start=True, stop=True)
                        ab = attp.tile([128, TH], bf16, tag="att", name="ab")
                        nc.scalar.activation(ab[:, lo:TH], pa[:, lo:TH], AF.Exp,
                                             scale=float(SCALE))
                        if sc >= q_lo:
                            nc.vector.tensor_mul(ab[:, lo:lo + 128],
                                                 ab[:, lo:lo + 128], mask[:])
                        att.append(ab)
                    py = pav.tile([HD + 1, TH], f32, tag="pav", name="py")
                    for si in range(n_si):
                        lo = max(si * 128, q_lo) - q_lo
                        nc.tensor.matmul(py[:, lo:TH],
                                         v_t[si][:, hh * 65:hh * 65 + 65],
                                         att[si][:, lo:TH],
                                         start=(si == 0), stop=(si == n_si - 1))
                    den_r = smp.tile([1, TH], f32, tag="sm", name="den")
                    nc.vector.reciprocal(den_r[:], py[HD:HD + 1, :])
                    den_b = dbp.tile([64, TH], f32, tag="db", name="den_b")
                    nc.gpsimd.partition_broadcast(den_b[:], den_r[:])
                    nc.vector.tensor_mul(
                        y_sb[hh // 2][th][(hh % 2) * 64:(hh % 2) * 64 + 64, :],
                        py[0:HD, :], den_b[:])

            def ychain_dmas(th):
                g_in = dr.tile([QO, TH], bf16, tag=f"gin{th}", name="gin")
                for i in range(2):
                    nc.sync.dma_start(g_in[i * 128:(i + 1) * 128, :],
                                      y_sb[i][th][:])
                g_out = dr.tile([C, TH], bf16, tag=f"gout{th}", name="gout")
                if collectives is True:
                    nc.gpsimd.collective_compute("AllGather", OP.bypass,
                                                 replica_groups=GROUPS,
                                                 ins=[g_in.opt()],
                                                 outs=[g_out.opt()])
                else:
                    for q in range(TP):
                        nc.sync.dma_start(g_out[q * QO:(q + 1) * QO, :], g_in[:])
                for cc in range(NCH):
                    yt = rbp.tile([128, TH], bf16, tag="rb", name="yt")
                    nc.sync.dma_start(yt[:], g_out[cc * 128:(cc + 1) * 128, :])
                    yts[th][cc] = yt

            def yresid(th, ccs=range(NCH)):
                for cc in ccs:
                    hb = hbp.tile([128, TH], bf16, tag="hb", name="hb")
                    nc.vector.tensor_add(hb[:], xt[cc][th][:], yts[th][cc][:])
                    h2[cc][th] = hb
                return lambda: [
                    nc.vector.tensor_add(xt[cc][th][:], xt[cc][th][:],
                                         yts[th][cc][:]) for cc in ccs]

            heads_phase(0)
            v_phase(range(4, 8))
            # prefetch next layer's qkv weights (wq slots free after V)
            if l + 1 < L:
                nxt_wq = load_wq(l + 1)
            ychain_dmas(0)
            xtu = [None, None]
            heads_phase(1, inject={3: lambda: xtu.__setitem__(
                0, yresid(0, range(4)))})
            xtu0b = yresid(0, range(4, 8))
            ychain_dmas(1)

            # ---- LN2 + MLP per phase; r-chain consumers issued late
            r_outs = [None, None]

            def mlp_phase(th, hoist=False, delay=0, post_stats=None):
                import contextlib as _cl2
                dp = tc.high_priority(offset=-delay) if delay else _cl2.nullcontext()
                with dp:
                    ln2 = ln_stats([h2[cc][th] for cc in range(NCH)],
                                   want_col=False)
                if post_stats:
                    post_stats()
                a_t = []
                import contextlib as _cl
                hp = tc.high_priority(offset=150) if hoist else _cl.nullcontext()
                with hp:
                  for fc in range(NCH):
                    pm = psb.tile([128, TH], f32, tag="psb", name="pm")
                    for cc in range(NCH):
                        nc.tensor.matmul(pm[:],
                                         w1_t[cc][:, fc * 128:(fc + 1) * 128],
                                         h2[cc][th][:],
                                         start=(cc == 0), stop=False)
                    nc.tensor.matmul(
                        pm[:], augw[0:1, 3 * QO + fc * 128:3 * QO + (fc + 1) * 128],
                        ln2["mu"][:], start=False, stop=zero_bias)
                    if not zero_bias:
                        nc.tensor.matmul(
                            pm[:],
                            augw[0:1,
                                 AW + 3 * QO + fc * 128:AW + 3 * QO + (fc + 1) * 128],
                            ln2["std"][:], start=False, stop=True)
                    nc.vector.tensor_mul(pm[:], pm[:], ln2["rstd_b"][:])
                    ga = gap.tile([128, TH], bf16, tag="ga", name="ga")
                    nc.scalar.activation(ga[:], pm[:], AF.Gelu)
                    a_t.append(ga)
                # end hoist

                r_in = dr.tile([128, NCH, TH], bf16, tag=f"rin{th}", name="rin")
                for cc in range(NCH):
                    pm2 = psb.tile([128, TH], f32, tag="psb", name="pm2")
                    for fc in range(NCH):
                        nc.tensor.matmul(pm2[:],
                                         w2_t[fc][:, cc * 128:(cc + 1) * 128],
                                         a_t[fc][:],
                                         start=(fc == 0), stop=(fc == NCH - 1))
                    rsb = rsp.tile([128, TH], bf16, tag="rs", name="rsb")
                    nc.scalar.activation(rsb[:], pm2[:], AF.Copy)
                    nc.scalar.dma_start(r_in[:, cc, :], rsb[:])
                r_out = dr.tile([128, NCH, TH], bf16, tag=f"rout{th}", name="rout")
                if collectives is True:
                    nc.gpsimd.collective_compute("AllReduce", OP.add,
                                                 replica_groups=GROUPS,
                                                 ins=[r_in.opt()],
                                                 outs=[r_out.opt()])
                else:
                    for qr in range(4):
                        nc.sync.dma_start(r_out[:, 2 * qr:2 * qr + 2, :],
                                          r_in[:, 2 * qr:2 * qr + 2, :])
                r_outs[th] = r_out

            r_ts = [[None] * 4, [None] * 4]

            def rread_dmas(th):
                r_out = r_outs[th]
                for half in range(4):
                    rt = rcp.tile([128, 2 * TH], bf16, tag="rc", name="rt")
                    nc.sync.dma_start(rt[:], r_out[:, half * 2:(half + 1) * 2, :])
                    r_ts[th][half] = rt

            def rresid(th):
                nxt = []
                for half in range(4):
                    rt = r_ts[th][half]
                    for q in range(2):
                        cc = half * 2 + q
                        rts = rt[:, q * TH:(q + 1) * TH]
                        b2col = b2c[:, l * 8 + cc:l * 8 + cc + 1]
                        t = xbp.tile([128, TH], bf16, tag="xb",
                                     name=f"xb{l}_{th}_{cc}")
                        nc.vector.scalar_tensor_tensor(
                            t[:], rts, b2col, xt[cc][th][:],
                            op0=OP.add, op1=OP.add)
                        xb[cc][th] = t
                        nxt.append(t)
                ln1[th] = ln_stats(nxt, want_col=True)
                for half in range(4):
                    rt = r_ts[th][half]
                    for q in range(2):
                        cc = half * 2 + q
                        nc.vector.scalar_tensor_tensor(
                            xt[cc][th][:], rt[:, q * TH:(q + 1) * TH],
                            b2c[:, l * 8 + cc:l * 8 + cc + 1], xt[cc][th][:],
                            op0=OP.add, op1=OP.add)

            mlp_phase(0, post_stats=lambda: (xtu[0](), xtu0b()))
            with tc.high_priority(offset=-60):
                xtu[1] = yresid(1)
            rread_dmas(0)
            mlp_phase(1, post_stats=lambda: xtu[1]())
            if l + 1 == L:
                hw_pre = []
                for half in range(2):
                    wt = hwp.tile([128, 4 * 512], bf16, tag="hw",
                                  name=f"hwp{half}")
                    nc.scalar.dma_start(
                        wt[:], hw_d[:, 0, half * 4:(half + 1) * 4, :])
                    hw_pre.append(wt)
            rread_dmas(1)
            if l + 1 < L:
                nxt_augw = load_augw(l + 1)
                nxt_w1 = load_w1(l + 1)
                nxt_w2 = load_w2(l + 1)
            rresid(0)
            rresid(1)
            if l + 1 < L:
                wq_t, augw = nxt_wq, nxt_augw
                w1_t, w2_t = nxt_w1, nxt_w2

        # ---- final LN: in-place bf16 centering of the xb copies + LM head
        hf = xb
        for th in range(2):
            mu_b = bcp.tile([128, TH], bf16, tag="bcmu")
            nc.gpsimd.partition_broadcast(mu_b[:], ln1[th]["mu"][:])
            for cc in range(NCH):
                nc.vector.tensor_sub(xb[cc][th][:], xb[cc][th][:], mu_b[:])

        for vb in range(NVB):
            if vb == 0:
                rhs = hw_pre
            else:
                rhs = []
                for half in range(2):
                    wt = hwp.tile([128, 4 * 512], bf16, tag="hw")
                    nc.scalar.dma_start(wt[:],
                                        hw_d[:, vb, half * 4:(half + 1) * 4, :])
                    rhs.append(wt)
            for tg in range(2):          # token groups of 4 tcc
                so = sop.tile([128, 4 * 512], bf16, tag="so")
                for ti in range(4):
                    tcc = tg * 4 + ti
                    th = tcc // 4
                    ph = psb.tile([128, 512], f32, tag="psb", name="ph")
                    for cc in range(NCH):
                        nc.tensor.matmul(
                            ph[:],
                            hf[cc][th][:, (tcc % 4) * 128:(tcc % 4 + 1) * 128],
                            rhs[cc // 4][:, (cc % 4) * 512:(cc % 4 + 1) * 512],
                            start=(cc == 0), stop=(cc == NCH - 1))
                    nc.scalar.activation(
                        so[:, ti * 512:(ti + 1) * 512], ph[:], AF.Copy,
                        scale=ln1[th]["col"][:, tcc % 4:tcc % 4 + 1])
                nc.scalar.dma_start(out_d[:, tg * 4:(tg + 1) * 4, vb, :], so[:])


def _prep_inputs(idx, tok_emb, pos_emb, ln1_w, ln1_b, wq, bq, wk, bk, wv, bv,
                 ln2_w, ln2_b, w1, b1, w2, b2, lnf_w, lnf_b, head_w):
    bf = ml_dtypes.bfloat16

    def cols128(a):  # [L, C] -> [128, L*8] per-partition column packing
        a = np.ascontiguousarray(a, np.float32)
        Lx = a.shape[0]
        return a.reshape(Lx, NCH, 128).transpose(2, 0, 1).reshape(128, Lx * NCH)

    mask = np.zeros((128, 128), np.float32)
    p, t = np.meshgrid(np.arange(128), np.arange(128), indexing="ij")
    mask[p <= t] = 1.0

    x0s = [np.ascontiguousarray(
        (tok_emb[np.asarray(idx[g], np.int64)] + pos_emb[0]).T, np.float32)
        for g in range(B)]
    x0bs = [x.astype(bf) for x in x0s]

    in_maps = []
    shard_cache = {}
    for c in range(8):
        g, j = c // 4, c % 4
        if j in shard_cache:
            m = dict(shard_cache[j])
            m["x0t"] = x0s[g]
            m["x0b"] = x0bs[g]
            in_maps.append(m)
            continue
        # gamma-folded weights, local slices
        wqkv_raw = np.concatenate(
            [wq[:, :, j * QO:(j + 1) * QO], wk[:, :, j * QO:(j + 1) * QO],
             wv[:, :, j * QO:(j + 1) * QO]], axis=2)          # [L, C, 768]
        wqkv_g = wqkv_raw * ln1_w[:, :, None]
        b_qkv = np.concatenate(
            [bq[:, j * QO:(j + 1) * QO], bk[:, j * QO:(j + 1) * QO],
             bv[:, j * QO:(j + 1) * QO]], axis=1)             # [L, 768]
        b_qkv_eff = b_qkv + np.einsum("lco,lc->lo", wqkv_raw, ln1_b)
        s_qkv = wqkv_g.sum(axis=1)                            # [L, 768]

        w1_raw = w1[:, :, j * FL:(j + 1) * FL]                # [L, C, FL]
        w1_g = w1_raw * ln2_w[:, :, None]
        b1_eff = b1[:, j * FL:(j + 1) * FL] + np.einsum("lcf,lc->lf", w1_raw, ln2_b)
        s_w1 = w1_g.sum(axis=1)

        augw = np.concatenate(
            [np.stack([-s_qkv, b_qkv_eff], axis=1),
             np.stack([-s_w1, b1_eff], axis=1)], axis=2)      # [L, 2, 768+FL]

        hw_g = head_w[:, j * VL:(j + 1) * VL] * lnf_w[:, None]  # [C, VL]
        hw_pad = np.zeros((C, VLP), np.float32)
        hw_pad[:, :VL] = hw_g
        # relayout: hw[p, vb, cc, q] = hw_pad[cc*128+p, vb*512+q]
        hw_r = hw_pad.reshape(NCH, 128, NVB, 512).transpose(1, 2, 0, 3)

        m = {
            "wqkv": np.ascontiguousarray(wqkv_g).astype(bf),
            "w1": np.ascontiguousarray(w1_g).astype(bf),
            "w2": np.ascontiguousarray(w2[:, j * FL:(j + 1) * FL, :]).astype(bf),
            "augw": np.ascontiguousarray(augw).astype(bf),
            "b2c": cols128(b2),
            "hw": np.ascontiguousarray(hw_r).astype(bf),
            "mask": mask.astype(bf),
        }
        shard_cache[j] = m
        m = dict(m)
        m["x0t"] = x0s[g]
        m["x0b"] = x0bs[g]
        in_maps.append(m)
    return in_maps


def kernel(**inputs):
    np_in = {k: np.asarray(v) for k, v in inputs.items()}
    zb = all(not np.any(np_in[k]) for k in
             ("bq", "bk", "bv", "b1", "ln1_b", "ln2_b"))
    key = f"nc_{zb}"
    if key not in _STATE:
        _STATE[key] = _build(zero_bias=zb)
    nc = _STATE[key]
    in_maps = _prep_inputs(**np_in)
    res = bass_utils.run_bass_kernel_spmd(nc, in_maps, core_ids=list(range(8)))
    outs = res.results
    # host epilogue: logits bias row from lnf_b (exact: out += lnf_b @ head_w)
    brow = (np.asarray(np_in["lnf_b"], np.float32) @
            np.asarray(np_in["head_w"], np.float32))          # [V]
    full = np.empty((B, T, V), np.float32)
    for c in range(8):
        g, j = c // 4, c % 4
        o = outs[c]["out"].astype(np.float32)                 # [128, NTC, NVB, 512]
        o = o.transpose(1, 0, 2, 3).reshape(T, VLP)[:, :VL]
        full[g, :, j * VL:(j + 1) * VL] = o + brow[j * VL:(j + 1) * VL]
    return full
